# revision 65
# baseline (speedup 1.0000x reference)
"""Trainium2 Bass kernel for nn_ASM_FineEnhancement (topk_masking).

Computation (per sample, B=4, x [256,256,256] f32):
  1. score all 256 coarse 16x16 patches: mean |x| over (C, 16, 16)
  2. top-64 patches by score
  3. per selected coarse patch, its 4 fine 8x8 patches get a per-patch
     3x3 conv (zero-padded per fine patch, 256->256 ch) + bias + relu
  4. output = x with enhanced patches scattered back

Sharding: the 4*64=256 selected coarse blocks and 4*192=768 unselected
blocks are distributed EVENLY over the 8 cores (32 conv + 96 stream
blocks each, exactly) - per-sample independence means any core can own
any block; the host reassembles. This removes all slot padding and all
inter-core imbalance, and makes every static shape input-independent
(one cached NEFF).

Per core the device sees
  xc [128, 2*16*584] fp8  conv blocks as vertical-mosaic strips
                          [p=ic_half, kc, group, 73rows*8cols]
  xs [128, 2*96*256] bf16 stream blocks [p, kc, blk, cell*64]
and emits
  oc [128, 16*2*584] bf16 enhanced patches (mosaic strips incl junk
                          separator rows; the host strips them)
  scores [1, 256]         per-block |x| half-sums (conv + stream)
The unselected blocks are READ by the device (scoring) but, like a
donated-buffer scatter, never written back: the host reuses the input
for the pass-through, which is therefore exact f32.

Conv pipeline (fp8 e4m3, weights x64 with 1/64 folded into the ACT
scale): per group the 8 fine cells stack vertically with one shared
zero row (cell c at strip rows 9c+1..9c+8). Cross-cell tap reads hit
genuine zeros (the per-patch padding), so EVERY 3x3 tap is a single
DoubleRow matmul contracting both 128-channel ic halves as k-tiles,
with a legal 4-dim AP [p, ktile, rows@8, cols@1]. PSUM holds each
(mc, strip-half) window; the full-coverage center tap goes first with
start=True. Junk outputs at separator rows ship in oc.

Stream: 1MB bf16 strips flow DRAM->SBUF (read-only); one DVE
abs-reduce per strip scores the blocks; two end-of-kernel
ones-matmuls fold partitions into a PSUM score row. PE (~72us of DoubleRow columns) and
DVE (~72us of reduces) are the co-critical engines; DMA ~56us.

The top-64 *selection* is computed on the host with the reference's
own eager jax-on-CPU ops: the rank-64/65 score gap can sit below fp32
resolution (sample 1 of the seed-0 input: true relative gap 1.1e-7),
so any on-device rescoring can disagree with the reference's
selection. The device still computes and emits all 256 block scores.
"""

import numpy as np

B, CH, H, W = 4, 256, 256, 256
CP, FP = 16, 8
K = 64                 # top-k coarse patches per sample
N_CORES = 8
NSEL = 32              # conv blocks per core (4*64/8)
NUNS = 96              # stream blocks per core (4*192/8)

_CACHE = {}

# tap order: full-coverage center tap first, so it is the start=True
# matmul of every PSUM accumulation group AND its weights are the first
# 512 wt columns (loaded by a small first DMA -> early PE start).
# The last tap must be dy==1 (DoubleRow) so it carries stop=True.
TAPS = [(1, 1), (0, 0), (0, 1), (0, 2), (2, 0), (2, 1), (2, 2),
        (1, 0), (1, 2)]
# stream strips: 1MB each, small at the end (the last reduce sits on
# the score-fold critical path)
STRIPS = [4096] * 11 + [2048, 1024, 1024]
# conv mosaic: per group (2 slots = 8 fine cells) the cells are stacked
# vertically with one shared zero row (cell c at rows 9c+1..9c+8), so
# every 3x3 tap is ONE kc-fused DoubleRow matmul with a legal 4-dim AP
# [p, ktile, rows@8, cols@1]. Junk outputs at the 9 separator rows are
# skipped by the output DMA. 73 rows x 8 cols = 584 elems per group.
GSTRIP = 584
# conv-input chunks (groups each): small first chunk so group 0's data
# lands early; wt pieces sized so tap 0's weights land even earlier
XCHUNKS = [1, 3, 4, 4, 4]
WPIECES = [512, 2048, 2048]


def _build(nrep=None):
    import concourse.bacc as bacc
    import concourse.mybir as mybir
    from concourse.tile import TileContext
    from concourse import bass

    F32 = mybir.dt.float32
    BF16 = mybir.dt.bfloat16
    FP8 = mybir.dt.float8e4

    CONVA = 2 * (NSEL // 2) * GSTRIP   # 18688 cols
    SAREA = 2 * NUNS * 256             # 49152 cols
    s_off = [0]
    for w in STRIPS:
        s_off.append(s_off[-1] + w)
    assert s_off[-1] == SAREA
    N_STRIP = len(STRIPS)

    nc = bacc.Bacc(None)
    xc = nc.declare_dram_parameter("xc", [128, CONVA], FP8, isOutput=False)
    xs = nc.declare_dram_parameter("xs", [128, SAREA], BF16, isOutput=False)
    wt = nc.declare_dram_parameter("wt", [128, 36 * 128], FP8, isOutput=False)
    bias = nc.declare_dram_parameter("bias", [128, 2], F32, isOutput=False)
    oc = nc.declare_dram_parameter("oc", [128, 32 * GSTRIP], BF16,
                                   isOutput=True)
    scores_out = nc.declare_dram_parameter("scores", [1, 256], F32,
                                           isOutput=True)

    # tap geometry: for tap offset d (0/1/2) along one axis, the valid
    # output window is [o0, o0+n) reading input rows [i0, i0+n)
    def clip(d):
        return (1, 0, 7) if d == 0 else ((0, 0, 8) if d == 1 else (0, 1, 7))

    from contextlib import ExitStack
    with TileContext(nc) as tc:
        _stk = ExitStack()
        if nrep:
            _stk.enter_context(tc.For_i(0, nrep))
        c_start = [0]
        for s in XCHUNKS:
            c_start.append(c_start[-1] + s)
        assert c_start[-1] == NSEL // 2
        grp2chunk = {}
        for g in range(NSEL // 2):
            for j in range(len(XCHUNKS)):
                if c_start[j] <= g < c_start[j + 1]:
                    grp2chunk[g] = j

        with tc.tile_pool(name="pers", bufs=1) as pers:
            wt_sb = pers.tile([128, 36 * 128], FP8, tag="wt")
            bias_sb = pers.tile([128, 2], F32, tag="bias")
            # conv input chunks: [p, kc, groups_j, 584] mosaic strips
            xcg = [pers.tile([128, 2 * s * GSTRIP], FP8, tag=f"xcg{j}",
                             name=f"xcg{j}") for j, s in enumerate(XCHUNKS)]
            cpart = pers.tile([128, 64], F32, tag="cpart")   # [kc, slot32]
            spart = pers.tile([128, 192], F32, tag="spart")
            scores_all = pers.tile([1, 256], F32, tag="sca")
            ones = pers.tile([128, 1], F32, tag="ones")
            cscr = pers.tile([128, 296], BF16, tag="cscr")  # Abs scratch

            # scores psum bank first, so the score folds do not wait
            # on the conv psum pool draining
            psc_cm = tc.tile_pool(name="psc", bufs=1, space="PSUM")
            psc = psc_cm.__enter__()

            # score row accumulator: [0:64] conv blocks, [64:256] stream
            ps4 = psc.tile([1, 256], F32, name="ps4")
            pOs_cm = tc.tile_pool(name="pOs", bufs=6)
            pOs = pOs_cm.__enter__()
            pA_cm = tc.tile_pool(name="pA", bufs=6)
            pA = pA_cm.__enter__()
            psum_cm = tc.tile_pool(name="psum", bufs=7, space="PSUM")
            psum_pool = psum_cm.__enter__()

            def emit_strip(i):
                lo, w = s_off[i], STRIPS[i]
                t = pA.tile([128, max(STRIPS)], BF16, tag="t", name="t")
                nc.sync.dma_start(out=t[:, :w], in_=xs[:, lo:lo + w])
                nc.vector.tensor_reduce(
                    out=spart[:, lo // 256:(lo + w) // 256],
                    in_=t[:, :w].rearrange("p (q c) -> p q c",
                                           q=w // 256, c=256),
                    axis=mybir.AxisListType.X, op=mybir.AluOpType.add,
                    apply_absolute_value=True)

            # queue policy: SP ring = all x traffic (xcg, strip in+out) so
            # the stream is DMA-paced, decoupled from conv progress; ACT
            # ring = wt/bias/oc/scores (conv-paced, small). The first two
            # strips are interleaved between the (small, fp8) xcg chunks
            # so the DVE's 72us reduce stream starts ~2us earlier.
            wo = 0
            for wpc in WPIECES:
                nc.scalar.dma_start(out=wt_sb[:, wo:wo + wpc],
                                    in_=wt[:, wo:wo + wpc])
                wo += wpc
            nc.scalar.dma_start(out=bias_sb[:], in_=bias[:])
            xap = xc[:]
            for j, s in enumerate(XCHUNKS):
                src = bass.AP(
                    tensor=xap.tensor,
                    offset=xap.offset + c_start[j] * GSTRIP,
                    ap=[[CONVA, 128], [CONVA // 2, 2], [1, s * GSTRIP]])
                nc.sync.dma_start(
                    out=xcg[j][:].rearrange("p (k e) -> p k e",
                                            k=2, e=s * GSTRIP),
                    in_=src)
                if j < 2:
                    emit_strip(j)
            nc.vector.memset(ones[:], 1.0)


            def emit_cred(j):
                # |x| half-sums of this chunk's slots (x 2 kc halves)
                # on DVE; zeros in the mosaic contribute nothing
                g0, gj = c_start[j], XCHUNKS[j]
                cp4 = cpart[:].rearrange("p (k s h) -> p k s h",
                                         k=2, s=16, h=2)
                inv = xcg[j][:].rearrange("p (k g e) -> p k g e",
                                          k=2, g=gj, e=GSTRIP)
                nc.vector.tensor_reduce(
                    out=cp4[:, :, g0:g0 + gj, 0], in_=inv[:, :, :, 0:296],
                    axis=mybir.AxisListType.X, op=mybir.AluOpType.add,
                    apply_absolute_value=True)
                nc.vector.tensor_reduce(
                    out=cp4[:, :, g0:g0 + gj, 1], in_=inv[:, :, :, 296:584],
                    axis=mybir.AxisListType.X, op=mybir.AluOpType.add,
                    apply_absolute_value=True)

            def emit_group(g):
                j = grp2chunk[g]
                lg = g - c_start[j]
                # [p, kc, group, 73 rows, 8 cols] mosaic strip view
                sv = xcg[j][:].rearrange("p (k g r c) -> p k g r c",
                                         k=2, g=XCHUNKS[j], r=73, c=8)
                # [p, ti, kc(2), mc, oc128]: weight pair view (DoubleRow)
                w4 = wt_sb[:].rearrange("p (t k m e) -> p t k m e",
                                        t=9, k=2, m=2, e=128)
                ost = pOs.tile([128, 2 * GSTRIP], BF16, tag="ost",
                               name="ost")
                # 4 psum tiles: (mc, h) with h = strip-row halves
                ps = [[psum_pool.tile([128, 512], F32, tag="ps", name="ps")
                       for _ in range(2)] for _ in range(2)]
                psv = [[p_[:].rearrange("p (r c) -> p r c", r=64, c=8)
                        for p_ in row] for row in ps]
                # every tap: ONE kc-fused DoubleRow matmul per (mc, h)
                for ti, (dy, dx) in enumerate(TAPS):
                    lo_v = 1 if dy == 0 else 0
                    hi_v = 71 if dy == 2 else 72
                    co, ci, ncc = clip(dx)
                    for h in range(2):
                        r0 = max(37 * h, lo_v)
                        r1 = min(36 + 36 * h, hi_v)
                        nr = r1 - r0 + 1
                        ri0 = r0 + dy - 1
                        rhs = sv[:, :, lg, ri0:ri0 + nr, ci:ci + ncc]
                        for mc in range(2):
                            nc.tensor.matmul(
                                psv[mc][h][:, r0 - 37 * h:
                                           r0 - 37 * h + nr,
                                           co:co + ncc],
                                lhsT=w4[:, ti, :, mc, :],
                                rhs=rhs,
                                start=(ti == 0), stop=(ti == 8),
                                perf_mode=mybir.MatmulPerfMode.DoubleRow)
                for mc in range(2):
                    for h in range(2):
                        nh = 37 - h
                        nc.scalar.activation(
                            ost[:, mc * GSTRIP + 296 * h:
                                mc * GSTRIP + 296 * h + nh * 8],
                            ps[mc][h][:, :nh * 8],
                            mybir.ActivationFunctionType.Relu,
                            bias=bias_sb[:, mc:mc + 1], scale=1.0 / 64)
                # contiguous write, separator junk rows included;
                # the host strips them during reassembly
                nc.scalar.dma_start(
                    out=oc[:, g * 2 * GSTRIP:(g + 1) * 2 * GSTRIP],
                    in_=ost[:])
            si = 2
            credded = set()
            for g in range(16):
                emit_group(g)
                j = grp2chunk[g]
                if j not in credded:
                    credded.add(j)
                    emit_cred(j)
                n_s = ((g + 1) * N_STRIP) // 16
                while si < n_s:
                    emit_strip(si)
                    si += 1
            while si < N_STRIP:
                emit_strip(si)
                si += 1
            # score folds: on HW the DMA-paced stream finishes well
            # before the conv, so these run right after the last conv
            # matmul without waiting
            nc.tensor.matmul(ps4[:, :64], lhsT=ones[:], rhs=cpart[:],
                             start=True, stop=False)
            nc.tensor.matmul(ps4[:, 64:], lhsT=ones[:], rhs=spart[:],
                             start=False, stop=True)

            psum_cm.__exit__(None, None, None)

            # ---- scores epilogue: row already folded in ps4
            nc.vector.tensor_copy(scores_all[:], ps4[:])
            nc.scalar.dma_start(out=scores_out[:], in_=scores_all[:])

            pA_cm.__exit__(None, None, None)
            pOs_cm.__exit__(None, None, None)
            psc_cm.__exit__(None, None, None)
        _stk.close()

    nc.finalize()
    return nc


def _host_selection(x):
    """Top-64 coarse patch indices per sample, bitwise-matching the
    reference (eager jax on CPU, same ops/order as reference.py)."""
    import jax
    cpu = jax.local_devices(backend="cpu")[0]
    import jax.numpy as jnp
    with jax.default_device(cpu):
        xj = jnp.asarray(x)
        Bb, C, Hh, Ww = xj.shape
        coarse = xj.reshape(Bb, C, 16, CP, 16, CP).transpose(
            0, 2, 4, 1, 3, 5).reshape(Bb, 256, C, CP, CP)
        scores = jnp.mean(jnp.abs(coarse), axis=(2, 3, 4))
        _, top_idx = jax.lax.top_k(scores, K)
        return np.asarray(top_idx)


def _blockize(x):
    """x [B,CH,H,W] -> [B, 256 blocks, CH, 256 elems] in cell layout
    (per block: 4 fine 8x8 patches row-major, each patch row-major)."""
    return np.ascontiguousarray(
        x.reshape(B, CH, 16, 2, 8, 16, 2, 8)
        .transpose(0, 2, 5, 1, 3, 6, 4, 7).reshape(B, 256, CH, 256))


def _unblockize(blocks):
    """Inverse of _blockize: [B, 256, CH, 256] -> [B, CH, H, W]."""
    return np.ascontiguousarray(
        blocks.reshape(B, 16, 16, CH, 2, 2, 8, 8)
        .transpose(0, 3, 1, 4, 6, 2, 5, 7).reshape(B, CH, H, W))


def _pack_plane(blk_sel, dtype=None):
    """[n, CH, 256] f32 -> [128, 2*n*256] dtype, [p, kc, n, 256]."""
    import ml_dtypes
    if dtype is None:
        dtype = ml_dtypes.bfloat16
    n = blk_sel.shape[0]
    arr = blk_sel.transpose(1, 0, 2).reshape(2, 128, n, 256) \
        .transpose(1, 0, 2, 3).reshape(128, 2 * n * 256)
    return np.ascontiguousarray(arr.astype(dtype))


def _pack_mosaic(blk_sel):
    """[32, CH, 256] f32 -> [128, 2*16*584] fp8 mosaic strips.

    Per group (2 slots = 8 cells) the cells sit at strip rows
    9c+1..9c+8 of a 73x8 strip; rows 9c are shared zero separators."""
    import ml_dtypes
    cells = blk_sel.reshape(16, 2, CH, 4, 8, 8) \
        .transpose(0, 2, 1, 3, 4, 5).reshape(16, CH, 8, 8, 8)
    strip = np.zeros((16, CH, 73, 8), np.float32)
    strip[:, :, 1:].reshape(16, CH, 8, 9, 8)[:, :, :, :8] = cells
    arr = strip.reshape(16, 2, 128, 584).transpose(2, 1, 0, 3) \
        .reshape(128, 2 * 16 * 584)
    return np.ascontiguousarray(arr.astype(ml_dtypes.float8_e4m3))


def _host_inputs(x, conv_w, conv_b):
    """Per-core input dicts + (sel, uns) block index lists."""
    x = np.asarray(x, np.float32)
    conv_w = np.asarray(conv_w, np.float32)
    conv_b = np.asarray(conv_b, np.float32)
    import ml_dtypes
    top_idx = _host_selection(x)
    # weights as lhsT blocks: wt[ic, ((ti*2+kc)*2+mc)*128+oc], ti = TAPS
    # order. fp8: x64 scale lifts them out of the e4m3 subnormal range;
    # the ACT epilogue divides it back out (scale=1/64).
    Wt = conv_w.transpose(1, 0, 2, 3) * 64.0  # [ic, oc, ky, kx]
    wt_host = np.empty((128, 36, 128), np.float32)
    for ti, (dy, dx) in enumerate(TAPS):
        for kc in range(2):
            for mc in range(2):
                wt_host[:, (ti * 2 + kc) * 2 + mc, :] = \
                    Wt[kc * 128:(kc + 1) * 128, mc * 128:(mc + 1) * 128,
                       dy, dx]
    wt_host = np.ascontiguousarray(
        wt_host.reshape(128, 36 * 128).astype(ml_dtypes.float8_e4m3))
    bias_host = np.ascontiguousarray(conv_b.reshape(2, 128).T)

    xb = _blockize(x)
    sel_s, sel_b, uns_s, uns_b = [], [], [], []
    for s in range(B):
        sel = np.sort(top_idx[s])
        uns = np.setdiff1d(np.arange(256), sel)
        sel_s.append(np.full(K, s)); sel_b.append(sel)
        uns_s.append(np.full(256 - K, s)); uns_b.append(uns)
    sel_s = np.concatenate(sel_s); sel_b = np.concatenate(sel_b)
    uns_s = np.concatenate(uns_s); uns_b = np.concatenate(uns_b)

    ins = []
    for c in range(N_CORES):
        cs, cb = sel_s[c * NSEL:(c + 1) * NSEL], sel_b[c * NSEL:(c + 1) * NSEL]
        us, ub = uns_s[c * NUNS:(c + 1) * NUNS], uns_b[c * NUNS:(c + 1) * NUNS]
        ins.append({
            "xc": _pack_mosaic(xb[cs, cb]),
            "xs": _pack_plane(xb[us, ub]),
            "wt": wt_host, "bias": bias_host,
        })
    return ins, (sel_s, sel_b, uns_s, uns_b)


def kernel(x, conv_w, conv_b):
    from concourse.bass_utils import run_bass_kernel_spmd
    ins, (sel_s, sel_b, uns_s, uns_b) = _host_inputs(x, conv_w, conv_b)
    if "nc" not in _CACHE:
        _CACHE["nc"] = _build()
    nc = _CACHE["nc"]
    res = run_bass_kernel_spmd(nc, ins, core_ids=list(range(N_CORES)))
    # pass-through blocks come straight from the (blockized) input; the
    # device reads them for scoring but, like a donated-buffer scatter,
    # only writes the enhanced patches back
    blocks = _blockize(np.asarray(x, np.float32))
    for c in range(N_CORES):
        cs, cb = sel_s[c * NSEL:(c + 1) * NSEL], sel_b[c * NSEL:(c + 1) * NSEL]
        o_c = res.results[c]["oc"].astype(np.float32) \
            .reshape(128, 16, 2, 73, 8)[:, :, :, 1:, :] \
            .reshape(128, 16, 2, 8, 9, 8)[:, :, :, :, :8, :] \
            .reshape(128, 16, 2, 2, 4, 8, 8) \
            .transpose(1, 3, 2, 0, 4, 5, 6).reshape(NSEL, CH, 256)
        blocks[cs, cb] = o_c
    return _unblockize(blocks)


# revision 73
# speedup vs baseline: 1.3873x; 1.3873x over previous
"""Trainium2 Bass kernel for nn_ASM_FineEnhancement (topk_masking).

Computation (per sample, B=4, x [256,256,256] f32):
  1. score all 256 coarse 16x16 patches: mean |x| over (C, 16, 16)
  2. top-64 patches by score
  3. per selected coarse patch, its 4 fine 8x8 patches get a per-patch
     3x3 conv (zero-padded per fine patch, 256->256 ch) + bias + relu
  4. output = x with enhanced patches scattered back

Sharding: the 4*64=256 selected coarse blocks and 4*192=768 unselected
blocks are distributed EVENLY over the 8 cores (32 conv + 96 stream
blocks each, exactly) - per-sample independence means any core can own
any block; the host reassembles. This removes all slot padding and all
inter-core imbalance, and makes every static shape input-independent
(one cached NEFF).

Per core the device sees
  xc [128, 2*16*584] fp8  conv blocks as vertical-mosaic strips
                          [p=ic_half, kc, group, 73rows*8cols]
  xs [128, 2*96*256] bf16 stream blocks [p, kc, blk, cell*64]
and emits
  oc [128, 16*2*584] bf16 enhanced patches (mosaic strips incl junk
                          separator rows; the host strips them)
  scores [1, 256]         per-block |x| half-sums (conv + stream)
The unselected blocks are READ by the device (scoring) but, like a
donated-buffer scatter, never written back: the host reuses the input
for the pass-through, which is therefore exact f32.

Conv pipeline (fp8 e4m3, weights x64 with 1/64 folded into the ACT
scale): per group the 8 fine cells stack vertically with one shared
zero row (cell c at strip rows 9c+1..9c+8). Cross-cell tap reads hit
genuine zeros (the per-patch padding), so EVERY 3x3 tap is a single
DoubleRow matmul contracting both 128-channel ic halves as k-tiles,
with a legal 4-dim AP [p, ktile, rows@8, cols@1]. PSUM holds each
(mc, strip-half) window; the full-coverage center tap goes first with
start=True. Junk outputs at separator rows ship in oc.

Stream: 1MB bf16 strips flow DRAM->SBUF (read-only); one DVE
abs-reduce per strip scores the blocks; two end-of-kernel
ones-matmuls fold partitions into a PSUM score row. PE (~72us of DoubleRow columns) and
DVE (~72us of reduces) are the co-critical engines; DMA ~56us.

The top-64 *selection* is computed on the host with the reference's
own eager jax-on-CPU ops: the rank-64/65 score gap can sit below fp32
resolution (sample 1 of the seed-0 input: true relative gap 1.1e-7),
so any on-device rescoring can disagree with the reference's
selection. The device still computes and emits all 256 block scores.
"""

import numpy as np

B, CH, H, W = 4, 256, 256, 256
CP, FP = 16, 8
K = 64                 # top-k coarse patches per sample
N_CORES = 8
NSEL = 32              # conv blocks per core (4*64/8)
NUNS = 96              # stream blocks per core (4*192/8)

_CACHE = {}

# tap order: full-coverage center tap first, so it is the start=True
# matmul of every PSUM accumulation group AND its weights are the first
# 512 wt columns (loaded by a small first DMA -> early PE start).
# The last tap must be dy==1 (DoubleRow) so it carries stop=True.
TAPS = [(1, 1), (0, 0), (0, 1), (0, 2), (2, 0), (2, 1), (2, 2),
        (1, 0), (1, 2)]
# stream strips: 1MB each, small at the end (the last reduce sits on
# the score-fold critical path)
STRIPS = [4096] * 11 + [2048] * 2
# conv mosaic: per group (2 slots = 8 fine cells) the cells are stacked
# vertically with one shared zero row (cell c at rows 9c+1..9c+8), so
# every 3x3 tap is ONE kc-fused DoubleRow matmul with a legal 4-dim AP
# [p, ktile, rows@8, cols@1]. Junk outputs at the 9 separator rows are
# skipped by the output DMA. 73 rows x 8 cols = 584 elems per group.
GSTRIP = 584
# conv-input chunks (groups each): small first chunk so group 0's data
# lands early; wt pieces sized so tap 0's weights land even earlier
XCHUNKS = [1, 3, 4, 4, 4]
WPIECES = [512, 2048, 2048]


def _build(nrep=None):
    import concourse.bacc as bacc
    import concourse.mybir as mybir
    from concourse.tile import TileContext
    from concourse import bass

    F32 = mybir.dt.float32
    BF16 = mybir.dt.bfloat16
    FP8 = mybir.dt.float8e4

    CONVA = 2 * (NSEL // 2) * GSTRIP   # 18688 cols
    SAREA = 2 * NUNS * 256             # 49152 cols
    s_off = [0]
    for w in STRIPS:
        s_off.append(s_off[-1] + w)
    assert s_off[-1] == SAREA
    N_STRIP = len(STRIPS)

    nc = bacc.Bacc(None)
    xc = nc.declare_dram_parameter("xc", [128, CONVA], FP8, isOutput=False)
    xs = nc.declare_dram_parameter("xs", [128, SAREA], BF16, isOutput=False)
    wt = nc.declare_dram_parameter("wt", [128, 36 * 128], FP8, isOutput=False)
    bias = nc.declare_dram_parameter("bias", [128, 2], F32, isOutput=False)
    oc = nc.declare_dram_parameter("oc", [128, 32 * GSTRIP], BF16,
                                   isOutput=True)
    scores_out = nc.declare_dram_parameter("scores", [1, 256], F32,
                                           isOutput=True)

    # tap geometry: for tap offset d (0/1/2) along one axis, the valid
    # output window is [o0, o0+n) reading input rows [i0, i0+n)
    def clip(d):
        return (1, 0, 7) if d == 0 else ((0, 0, 8) if d == 1 else (0, 1, 7))

    from contextlib import ExitStack
    with TileContext(nc) as tc:
        _stk = ExitStack()
        if nrep:
            _stk.enter_context(tc.For_i(0, nrep))
        c_start = [0]
        for s in XCHUNKS:
            c_start.append(c_start[-1] + s)
        assert c_start[-1] == NSEL // 2
        grp2chunk = {}
        for g in range(NSEL // 2):
            for j in range(len(XCHUNKS)):
                if c_start[j] <= g < c_start[j + 1]:
                    grp2chunk[g] = j

        with tc.tile_pool(name="pers", bufs=1) as pers:
            wt_sb = pers.tile([128, 36 * 128], FP8, tag="wt")
            bias_sb = pers.tile([128, 2], F32, tag="bias")
            # conv input chunks: [p, kc, groups_j, 584] mosaic strips
            xcg = [pers.tile([128, 2 * s * GSTRIP], FP8, tag=f"xcg{j}",
                             name=f"xcg{j}") for j, s in enumerate(XCHUNKS)]
            cpart = pers.tile([128, 64], F32, tag="cpart")   # [kc, slot32]
            spart = pers.tile([128, 192], F32, tag="spart")
            scores_all = pers.tile([1, 256], F32, tag="sca")
            ones = pers.tile([128, 1], F32, tag="ones")
            cscr = pers.tile([128, 296], BF16, tag="cscr")  # Abs scratch

            # scores psum bank first, so the score folds do not wait
            # on the conv psum pool draining
            psc_cm = tc.tile_pool(name="psc", bufs=1, space="PSUM")
            psc = psc_cm.__enter__()

            # score row accumulator: [0:64] conv blocks, [64:256] stream
            ps4 = psc.tile([1, 256], F32, name="ps4")
            pOs_cm = tc.tile_pool(name="pOs", bufs=6)
            pOs = pOs_cm.__enter__()
            pA_cm = tc.tile_pool(name="pA", bufs=6)
            pA = pA_cm.__enter__()
            psum_cm = tc.tile_pool(name="psum", bufs=7, space="PSUM")
            psum_pool = psum_cm.__enter__()

            def emit_strip(i):
                lo, w = s_off[i], STRIPS[i]
                t = pA.tile([128, max(STRIPS)], BF16, tag="t", name="t")
                nc.sync.dma_start(out=t[:, :w], in_=xs[:, lo:lo + w])
                nc.vector.tensor_reduce(
                    out=spart[:, lo // 256:(lo + w) // 256],
                    in_=t[:, :w].rearrange("p (q c) -> p q c",
                                           q=w // 256, c=256),
                    axis=mybir.AxisListType.X, op=mybir.AluOpType.add,
                    apply_absolute_value=True)

            # queue policy: SP ring = all x traffic (xcg, strip in+out) so
            # the stream is DMA-paced, decoupled from conv progress; ACT
            # ring = wt/bias/oc/scores (conv-paced, small). No strips before
            # the xcg chunks: 1MB strips on the shared pipe delay the
            # wt pieces and xcg chunks the PE needs in its first 10us
            # (sim: +4us of early Ldweights stalls when interleaved).
            wo = 0
            for wpc in WPIECES:
                nc.scalar.dma_start(out=wt_sb[:, wo:wo + wpc],
                                    in_=wt[:, wo:wo + wpc])
                wo += wpc
            nc.scalar.dma_start(out=bias_sb[:], in_=bias[:])
            xap = xc[:]
            for j, s in enumerate(XCHUNKS):
                src = bass.AP(
                    tensor=xap.tensor,
                    offset=xap.offset + c_start[j] * GSTRIP,
                    ap=[[CONVA, 128], [CONVA // 2, 2], [1, s * GSTRIP]])
                # chunks 2+ ride the ACT ring BEHIND the wt pieces: the
                # per-ring FIFO then prioritizes the weights (needed by
                # PE within ~8us) over chunks PE won't touch for 20us+,
                # instead of the engine round-robin splitting bandwidth
                eng = nc.sync if j < 2 else nc.scalar
                eng.dma_start(
                    out=xcg[j][:].rearrange("p (k e) -> p k e",
                                            k=2, e=s * GSTRIP),
                    in_=src)

            nc.vector.memset(ones[:], 1.0)


            def emit_cred(j):
                # |x| half-sums of this chunk's slots (x 2 kc halves)
                # on DVE; zeros in the mosaic contribute nothing
                g0, gj = c_start[j], XCHUNKS[j]
                cp4 = cpart[:].rearrange("p (k s h) -> p k s h",
                                         k=2, s=16, h=2)
                inv = xcg[j][:].rearrange("p (k g e) -> p k g e",
                                          k=2, g=gj, e=GSTRIP)
                nc.vector.tensor_reduce(
                    out=cp4[:, :, g0:g0 + gj, 0], in_=inv[:, :, :, 0:296],
                    axis=mybir.AxisListType.X, op=mybir.AluOpType.add,
                    apply_absolute_value=True)
                nc.vector.tensor_reduce(
                    out=cp4[:, :, g0:g0 + gj, 1], in_=inv[:, :, :, 296:584],
                    axis=mybir.AxisListType.X, op=mybir.AluOpType.add,
                    apply_absolute_value=True)

            def emit_group(g):
                j = grp2chunk[g]
                lg = g - c_start[j]
                # [p, kc, group, 73 rows, 8 cols] mosaic strip view
                sv = xcg[j][:].rearrange("p (k g r c) -> p k g r c",
                                         k=2, g=XCHUNKS[j], r=73, c=8)
                # [p, ti, kc(2), mc, oc128]: weight pair view (DoubleRow)
                w4 = wt_sb[:].rearrange("p (t k m e) -> p t k m e",
                                        t=9, k=2, m=2, e=128)
                ost = pOs.tile([128, 2 * GSTRIP], BF16, tag="ost",
                               name="ost")
                # 4 psum tiles: (mc, h) with h = strip-row halves
                ps = [[psum_pool.tile([128, 512], F32, tag="ps", name="ps")
                       for _ in range(2)] for _ in range(2)]
                psv = [[p_[:].rearrange("p (r c) -> p r c", r=64, c=8)
                        for p_ in row] for row in ps]
                # every tap: ONE kc-fused DoubleRow matmul per (mc, h)
                for ti, (dy, dx) in enumerate(TAPS):
                    lo_v = 1 if dy == 0 else 0
                    hi_v = 71 if dy == 2 else 72
                    co, ci, ncc = clip(dx)
                    for h in range(2):
                        r0 = max(37 * h, lo_v)
                        r1 = min(36 + 36 * h, hi_v)
                        nr = r1 - r0 + 1
                        ri0 = r0 + dy - 1
                        rhs = sv[:, :, lg, ri0:ri0 + nr, ci:ci + ncc]
                        for mc in range(2):
                            nc.tensor.matmul(
                                psv[mc][h][:, r0 - 37 * h:
                                           r0 - 37 * h + nr,
                                           co:co + ncc],
                                lhsT=w4[:, ti, :, mc, :],
                                rhs=rhs,
                                start=(ti == 0), stop=(ti == 8),
                                perf_mode=mybir.MatmulPerfMode.DoubleRow)
                for mc in range(2):
                    for h in range(2):
                        nh = 37 - h
                        nc.scalar.activation(
                            ost[:, mc * GSTRIP + 296 * h:
                                mc * GSTRIP + 296 * h + nh * 8],
                            ps[mc][h][:, :nh * 8],
                            mybir.ActivationFunctionType.Relu,
                            bias=bias_sb[:, mc:mc + 1], scale=1.0 / 64)
                # contiguous write, separator junk rows included;
                # the host strips them during reassembly
                nc.scalar.dma_start(
                    out=oc[:, g * 2 * GSTRIP:(g + 1) * 2 * GSTRIP],
                    in_=ost[:])
            si = 0
            credded = set()
            for g in range(16):
                emit_group(g)
                j = grp2chunk[g]
                if j not in credded:
                    credded.add(j)
                    emit_cred(j)
                n_s = ((g + 1) * N_STRIP) // 16
                while si < n_s:
                    emit_strip(si)
                    si += 1
            while si < N_STRIP:
                emit_strip(si)
                si += 1
            # score folds: on HW the DMA-paced stream finishes well
            # before the conv, so these run right after the last conv
            # matmul without waiting
            nc.tensor.matmul(ps4[:, :64], lhsT=ones[:], rhs=cpart[:],
                             start=True, stop=False)
            nc.tensor.matmul(ps4[:, 64:], lhsT=ones[:], rhs=spart[:],
                             start=False, stop=True)

            psum_cm.__exit__(None, None, None)

            # ---- scores epilogue: row already folded in ps4
            nc.vector.tensor_copy(scores_all[:], ps4[:])
            nc.scalar.dma_start(out=scores_out[:], in_=scores_all[:])

            pA_cm.__exit__(None, None, None)
            pOs_cm.__exit__(None, None, None)
            psc_cm.__exit__(None, None, None)
        _stk.close()

    nc.finalize()
    return nc


def _host_selection(x):
    """Top-64 coarse patch indices per sample, bitwise-matching the
    reference (eager jax on CPU, same ops/order as reference.py)."""
    import jax
    cpu = jax.local_devices(backend="cpu")[0]
    import jax.numpy as jnp
    with jax.default_device(cpu):
        xj = jnp.asarray(x)
        Bb, C, Hh, Ww = xj.shape
        coarse = xj.reshape(Bb, C, 16, CP, 16, CP).transpose(
            0, 2, 4, 1, 3, 5).reshape(Bb, 256, C, CP, CP)
        scores = jnp.mean(jnp.abs(coarse), axis=(2, 3, 4))
        _, top_idx = jax.lax.top_k(scores, K)
        return np.asarray(top_idx)


def _blockize(x):
    """x [B,CH,H,W] -> [B, 256 blocks, CH, 256 elems] in cell layout
    (per block: 4 fine 8x8 patches row-major, each patch row-major)."""
    return np.ascontiguousarray(
        x.reshape(B, CH, 16, 2, 8, 16, 2, 8)
        .transpose(0, 2, 5, 1, 3, 6, 4, 7).reshape(B, 256, CH, 256))


def _unblockize(blocks):
    """Inverse of _blockize: [B, 256, CH, 256] -> [B, CH, H, W]."""
    return np.ascontiguousarray(
        blocks.reshape(B, 16, 16, CH, 2, 2, 8, 8)
        .transpose(0, 3, 1, 4, 6, 2, 5, 7).reshape(B, CH, H, W))


def _pack_plane(blk_sel, dtype=None):
    """[n, CH, 256] f32 -> [128, 2*n*256] dtype, [p, kc, n, 256]."""
    import ml_dtypes
    if dtype is None:
        dtype = ml_dtypes.bfloat16
    n = blk_sel.shape[0]
    arr = blk_sel.transpose(1, 0, 2).reshape(2, 128, n, 256) \
        .transpose(1, 0, 2, 3).reshape(128, 2 * n * 256)
    return np.ascontiguousarray(arr.astype(dtype))


def _pack_mosaic(blk_sel):
    """[32, CH, 256] f32 -> [128, 2*16*584] fp8 mosaic strips.

    Per group (2 slots = 8 cells) the cells sit at strip rows
    9c+1..9c+8 of a 73x8 strip; rows 9c are shared zero separators."""
    import ml_dtypes
    cells = blk_sel.reshape(16, 2, CH, 4, 8, 8) \
        .transpose(0, 2, 1, 3, 4, 5).reshape(16, CH, 8, 8, 8)
    strip = np.zeros((16, CH, 73, 8), np.float32)
    strip[:, :, 1:].reshape(16, CH, 8, 9, 8)[:, :, :, :8] = cells
    arr = strip.reshape(16, 2, 128, 584).transpose(2, 1, 0, 3) \
        .reshape(128, 2 * 16 * 584)
    return np.ascontiguousarray(arr.astype(ml_dtypes.float8_e4m3))


def _host_inputs(x, conv_w, conv_b):
    """Per-core input dicts + (sel, uns) block index lists."""
    x = np.asarray(x, np.float32)
    conv_w = np.asarray(conv_w, np.float32)
    conv_b = np.asarray(conv_b, np.float32)
    import ml_dtypes
    top_idx = _host_selection(x)
    # weights as lhsT blocks: wt[ic, ((ti*2+kc)*2+mc)*128+oc], ti = TAPS
    # order. fp8: x64 scale lifts them out of the e4m3 subnormal range;
    # the ACT epilogue divides it back out (scale=1/64).
    Wt = conv_w.transpose(1, 0, 2, 3) * 64.0  # [ic, oc, ky, kx]
    wt_host = np.empty((128, 36, 128), np.float32)
    for ti, (dy, dx) in enumerate(TAPS):
        for kc in range(2):
            for mc in range(2):
                wt_host[:, (ti * 2 + kc) * 2 + mc, :] = \
                    Wt[kc * 128:(kc + 1) * 128, mc * 128:(mc + 1) * 128,
                       dy, dx]
    wt_host = np.ascontiguousarray(
        wt_host.reshape(128, 36 * 128).astype(ml_dtypes.float8_e4m3))
    bias_host = np.ascontiguousarray(conv_b.reshape(2, 128).T)

    xb = _blockize(x)
    sel_s, sel_b, uns_s, uns_b = [], [], [], []
    for s in range(B):
        sel = np.sort(top_idx[s])
        uns = np.setdiff1d(np.arange(256), sel)
        sel_s.append(np.full(K, s)); sel_b.append(sel)
        uns_s.append(np.full(256 - K, s)); uns_b.append(uns)
    sel_s = np.concatenate(sel_s); sel_b = np.concatenate(sel_b)
    uns_s = np.concatenate(uns_s); uns_b = np.concatenate(uns_b)

    ins = []
    for c in range(N_CORES):
        cs, cb = sel_s[c * NSEL:(c + 1) * NSEL], sel_b[c * NSEL:(c + 1) * NSEL]
        us, ub = uns_s[c * NUNS:(c + 1) * NUNS], uns_b[c * NUNS:(c + 1) * NUNS]
        ins.append({
            "xc": _pack_mosaic(xb[cs, cb]),
            "xs": _pack_plane(xb[us, ub]),
            "wt": wt_host, "bias": bias_host,
        })
    return ins, (sel_s, sel_b, uns_s, uns_b)


def kernel(x, conv_w, conv_b):
    from concourse.bass_utils import run_bass_kernel_spmd
    ins, (sel_s, sel_b, uns_s, uns_b) = _host_inputs(x, conv_w, conv_b)
    if "nc" not in _CACHE:
        _CACHE["nc"] = _build()
    nc = _CACHE["nc"]
    res = run_bass_kernel_spmd(nc, ins, core_ids=list(range(N_CORES)))
    # pass-through blocks come straight from the (blockized) input; the
    # device reads them for scoring but, like a donated-buffer scatter,
    # only writes the enhanced patches back
    blocks = _blockize(np.asarray(x, np.float32))
    for c in range(N_CORES):
        cs, cb = sel_s[c * NSEL:(c + 1) * NSEL], sel_b[c * NSEL:(c + 1) * NSEL]
        o_c = res.results[c]["oc"].astype(np.float32) \
            .reshape(128, 16, 2, 73, 8)[:, :, :, 1:, :] \
            .reshape(128, 16, 2, 8, 9, 8)[:, :, :, :, :8, :] \
            .reshape(128, 16, 2, 2, 4, 8, 8) \
            .transpose(1, 3, 2, 0, 4, 5, 6).reshape(NSEL, CH, 256)
        blocks[cs, cb] = o_c
    return _unblockize(blocks)
